# revision 8
# baseline (speedup 1.0000x reference)
"""Trainium2 Bass kernel for nn_Block_3951369912372 (dense transformer block).

Reference computation (per batch element b of 8, handled by core b):
  x: [4, 512, 768]  (S=4 groups of N=512 tokens, D=768)
  h   = LN(x; g1, b1)
  qkv = h @ Wqkv                      (12 heads, head_dim 64)
  attn over the N=512 tokens within each s-group, per head
  y   = attn_out @ Wp + bp;  x2 = x + y
  h2  = LN(x2; g2, b2)
  out = x2 + gelu(h2 @ W1 + bm1) @ W2 + bm2

v2: the attention path (qkv, V, attn@V, proj) runs in fp8 e4m3 with
DoubleRow perf mode (2 k-slabs per matmul); weights are pre-scaled x32 on
the host so they clear the e4m3 subnormal band, and the 1/32 is folded
into the PSUM-evacuation ops.  The MLP stays bf16 (fp8 there costs ~2e-2
rel err, over budget).  W1 is SBUF-resident; only W2 streams.  x, x2 and
all LN inputs are bf16, so LN stats matmuls need no scalar pre-copies.
Softmax: exp(s - 2) written straight to fp8, denominators from a ones
column in V, batched per block into one [12, 512] tile and inverted with
a scalar-engine ln->exp chain (walrus here can't encode the custom-DVE
fast reciprocal and the DVE reciprocal costs 3.3us a row).  All
dependent row-broadcast DMAs ride the gpsimd queue; the sync queue only
carries independent bulk streams (x in, out, W2 tiles).
"""

import numpy as np
import ml_dtypes

import bass_rust
import concourse.bass as bass
import concourse.mybir as mybir
import concourse.tile as tile
from concourse.bass_utils import run_bass_kernel_spmd

F32 = mybir.dt.float32
BF16 = mybir.dt.bfloat16
F8 = mybir.dt.float8e4
AF = mybir.ActivationFunctionType
OP = mybir.AluOpType
DR = mybir.MatmulPerfMode.DoubleRow

DIM = 768
HEADS = 12
HD = 64
HIDDEN = 3072
EPS = 1e-5
SCALE = HD ** -0.5
S = 4
N = 512
TOK = S * N          # tokens per core
P = 128
NCH = DIM // P       # 6 feature chunks
KCH = HIDDEN // P    # 24 hidden chunks
NPAIR = HEADS // 2   # 6 head pairs
WS = 32.0            # fp8 weight pre-scale
VW = 80              # v-tile row stride (64 feats + ones col + pad to x16)
ESH = 2.0            # exp shift: softmax uses exp(s - ESH), cancels in the sum


class FixedTileContext(tile.TileContext):
    """Walrus in this container rejects instructions with more than ~1 sem
    wait ("Too many sync wait commands").  After scheduling, spread excess
    waits onto same-engine carrier nops inserted immediately before the
    over-limit instruction."""

    CAP = 1

    def _drain_and_barrier(self, tick_clock, wait_clock):
        super()._drain_and_barrier(tick_clock, wait_clock)
        nc = self.nc
        # Drop Ldweights that reload the exact weights already resident in
        # the PE array (same AP/config as the immediately preceding
        # Ldweights, with no sync side effects).
        for bb in nc.m.functions[0].blocks:
            prev_key = None
            newlist = []
            for inst in bb.instructions:
                tn = type(inst).__name__
                if tn == "InstLdweights":
                    si = inst.sync_info
                    clean = si is None or (not si.on_wait and not si.on_update)
                    key = (str(inst.ins),
                           str(getattr(inst, "perf_mode", None)),
                           str(getattr(inst, "is_transpose", None)),
                           str(getattr(inst, "tile_position", None)))
                    if clean and key == prev_key:
                        continue  # redundant reload
                    if clean:
                        prev_key = key
                    else:
                        prev_key = None
                elif tn not in ("InstMatmult",):
                    pass  # non-PE instructions don't disturb PE weights
                newlist.append(inst)
            bb.instructions = newlist
        nfix = 0
        for bb in nc.m.functions[0].blocks:
            newlist = []
            changed = False
            for inst in bb.instructions:
                si = inst.sync_info
                waits = list(si.on_wait) if si is not None else []
                if len(waits) > self.CAP:
                    for w in waits[:-self.CAP]:
                        nop = mybir.InstNoOp(
                            name=f"I-waitfix-{nfix}",
                            sync_info=bass_rust.SyncInfo(
                                on_wait=[w], on_update=[]),
                            bass_nofuse=True,
                            engine=inst.engine,
                        )
                        nfix += 1
                        nc.register_instruction(nop, overwrite=True)
                        newlist.append(nop)
                    si.on_wait = waits[-self.CAP:]
                    changed = True
                newlist.append(inst)
            if changed:
                bb.instructions = newlist


def build_bass(debug=False, reps=1):
    nc = bass.Bass("TRN2", target_bir_lowering=False, debug=False, num_devices=8)

    xT = nc.dram_tensor("xT", [DIM, TOK], BF16, kind="ExternalInput")
    # fp8 attention weights, [p, chunk, cols] layout, pre-scaled x32
    wq = nc.dram_tensor("wq", [P, NCH, DIM], F8, kind="ExternalInput")
    wk = nc.dram_tensor("wk", [P, NCH, DIM], F8, kind="ExternalInput")
    wv = nc.dram_tensor("wv", [P, NCH, DIM], F8, kind="ExternalInput")
    wp = nc.dram_tensor("wp", [P, NCH, DIM], F8, kind="ExternalInput")
    # W1 streamed pre-tiled (each [P,3,NCH,P] group contiguous); W2 resident
    w1 = nc.dram_tensor("w1", [KCH, P, NCH, P], BF16, kind="ExternalInput")
    w2 = nc.dram_tensor("w2", [P, KCH, DIM], BF16, kind="ExternalInput")
    # per-feature biases, [p, chunk] layout
    bq = nc.dram_tensor("bq", [P, NCH], F32, kind="ExternalInput")
    bk = nc.dram_tensor("bk", [P, NCH], F32, kind="ExternalInput")
    bm1 = nc.dram_tensor("bm1", [P, KCH], F32, kind="ExternalInput")
    bm2 = nc.dram_tensor("bm2", [P, NCH], F32, kind="ExternalInput")

    out = nc.dram_tensor("out", [DIM, TOK], F32, kind="ExternalOutput")

    xT_a = xT.ap().rearrange("(c p) t -> p c t", p=P)
    out_a = out.ap().rearrange("(c p) t -> p c t", p=P)

    with FixedTileContext(nc) as tc:
        args = (nc, tc, xT_a, out_a,
                wq.ap(), wk.ap(), wv.ap(), wp.ap(), w1.ap(), w2.ap(),
                bq.ap(), bk.ap(), bm1.ap(), bm2.ap())
        for _ in range(reps):
            _body(*args)
    return nc


PHASE_LOG = []


def _body(nc, tc, xT_a, out_a, wq_a, wk_a, wv_a, wp_a, w1_a, w2_a,
          bq_a, bk_a, bm1_a, bm2_a):
    ctx_pools = {}

    def mark(name):
        PHASE_LOG.append((name, nc.get_next_instruction_name()))

    def pool(name, bufs, space="SBUF"):
        p = tc.alloc_tile_pool(name=name, bufs=bufs, space=space)
        ctx_pools[name] = p
        return p

    # ---- persistent (bufs=1) ----
    singles = pool("singles", 1)
    wq_sb = singles.tile([P, NCH, DIM], F8, tag="wq")
    wk_sb = singles.tile([P, NCH, DIM], F8, tag="wk")
    wv_sb = singles.tile([P, NCH, DIM], F8, tag="wv")
    wp_sb = singles.tile([P, NCH, DIM], F8, tag="wp")
    w2_sb = singles.tile([P, KCH, DIM], BF16, tag="w2")
    bq_sb = singles.tile([P, NCH], F32, tag="bq")
    bk_sb = singles.tile([P, NCH], F32, tag="bk")
    bm1_sb = singles.tile([P, KCH], F32, tag="bm1")
    bm2_sb = singles.tile([P, NCH], F32, tag="bm2")
    nc.sync.dma_start(bq_sb[:], bq_a)
    nc.sync.dma_start(bk_sb[:], bk_a)
    nc.sync.dma_start(bm1_sb[:], bm1_a)
    nc.sync.dma_start(bm2_sb[:], bm2_a)
    ones_col = singles.tile([P, 1], BF16, tag="ones")
    nc.vector.memset(ones_col[:], 1.0)
    eps_tile = singles.tile([1, 1], F32, tag="eps")
    nc.vector.memset(eps_tile[:], EPS)
    zero_col = singles.tile([P, 1], F32, tag="zero")
    nc.vector.memset(zero_col[:], 0.0)
    neg2_col = singles.tile([P, 1], F32, tag="neg2")
    nc.vector.memset(neg2_col[:], -ESH)

    # ---- pools ----
    xT_p = pool("xT", 2)          # [P, NCH, N] bf16
    xb_p = pool("xb", 2)          # [P, N] bf16 squares for LN stats
    rows_p = pool("rows", 7)      # [1, N] stat rows
    bcast_p = pool("bcast", 1)    # [P, 2, N] bf16 LN broadcast rows
    nrm_p = pool("nrm", 1)        # bf16 LN1 normalize staging (fp8 RMW dodge)
    hT_p = pool("hT", 2)          # [P, NCH, N] fp8
    v_p = pool("V", 2)            # [P, 2, HEADS, VW] fp8 per tok-tile-pair
    qk_p = pool("qk", 3)          # [P, N] bf16 (q-pair / k-pair)
    pt_p = pool("PT", 2)          # [P, 4, N] fp8 exp(scores^T - 2)
    ds_p = pool("dstage", 2)      # [HEADS, N] denominator staging rows
    rb_p = pool("rb", 1)          # [P, NCH, N] bf16 recip broadcast map
    yraw_p = pool("yraw", 1)      # [P, NCH, N] bf16 unnormalized attn out
    yT_p = pool("yT", 2)          # [P, NCH, N] fp8
    x2T_p = pool("x2T", 2)        # [P, NCH, N] bf16
    h2T_p = pool("h2T", 1)        # [P, NCH, N] bf16
    w1_p = pool("w1t", 2)         # [P, 3, NCH, P] streamed W1 tile
    mT_p = pool("mT", 1)          # [P, KCH, N] bf16
    outp = pool("outT", 2)        # [P, N] f32
    # PSUM banks: scores 3x[128,512] + mm 3x[128,512] + acc 2x[128,512] = 8
    ps_sc = pool("ps_sc", 3, space="PSUM")
    ps_mm = pool("ps_mm", 3, space="PSUM")
    ps_acc = pool("ps_acc", 2, space="PSUM")
    dram_p = pool("drows", 6, space="DRAM")

    def layernorm(src_tile, dst_pool, dst_dtype):
        """src [P, NCH, N] bf16 -> normalized [P, NCH, N] (no affine)."""
        st = ps_mm.tile([P, N], F32, tag="mm")
        st2 = ps_mm.tile([P, N], F32, tag="mm")
        for c in range(NCH):
            nc.tensor.matmul(st[0:1, :], ones_col[:], src_tile[:, c, :],
                             start=(c == 0), stop=(c == NCH - 1))
            sq = xb_p.tile([P, N], BF16, tag="xb")
            nc.vector.tensor_tensor(sq[:], src_tile[:, c, :], src_tile[:, c, :],
                                    OP.mult)
            nc.tensor.matmul(st2[0:1, :], ones_col[:], sq[:],
                             start=(c == 0), stop=(c == NCH - 1))
        mu = rows_p.tile([1, N], F32, tag="row")
        nc.vector.tensor_scalar_mul(mu[:], st[0:1, :], 1.0 / DIM)
        mu2 = rows_p.tile([1, N], F32, tag="row")
        nc.vector.tensor_tensor(mu2[:], mu[:], mu[:], OP.mult)
        var = rows_p.tile([1, N], F32, tag="row")
        nc.vector.scalar_tensor_tensor(var[:], st2[0:1, :], 1.0 / DIM,
                                       mu2[:], OP.mult, OP.subtract)
        # rstd = exp(-0.5 * ln(var + eps)); scalar Rsqrt/Reciprocal are
        # gated off in bass, the DVE reciprocal costs 3.3us a row.
        lnv = rows_p.tile([1, N], F32, tag="row")
        nc.scalar.activation(lnv[:], var[:], AF.Ln, bias=eps_tile[:])
        rstd = rows_p.tile([1, N], BF16, tag="rowb")
        nc.scalar.activation(rstd[:], lnv[:], AF.Exp, scale=-0.5,
                             bias=zero_col[0:1, :])
        nmr = rows_p.tile([1, N], BF16, tag="rowb")
        nc.vector.scalar_tensor_tensor(nmr[:], mu[:], -1.0, rstd[:],
                                       OP.mult, OP.mult)
        dr = dram_p.tile([1, 2 * N], BF16, tag="dr2")
        nc.gpsimd.dma_start(dr[:, 0:N], rstd[:])
        nc.gpsimd.dma_start(dr[:, N:2 * N], nmr[:])
        bt = bcast_p.tile([P, 2, N], BF16, tag="bc")
        nc.gpsimd.dma_start(bt[:], dr[:].rearrange("o (x t) -> o x t", t=N)
                            .to_broadcast((P, 2, N)))
        rstd_b = bt[:, 0, :]
        nmr_b = bt[:, 1, :]
        dst = dst_pool.tile([P, NCH, N], dst_dtype, tag="h")
        # gpsimd's tensor_tensor is ~2.6x slower than the DVE's: 4/2 split.
        # For fp8 dst, stage the intermediate in bf16: a fp8 read-modify-write
        # op measures 4.1us on the DVE vs 1.8us for the all-bf16 one; with the
        # staging every op only ever WRITES fp8, never reads it.
        for eng, lo, hi in ((nc.vector, 0, 4), (nc.gpsimd, 4, NCH)):
            if dst_dtype == F8:
                mid = nrm_p.tile([P, hi - lo, N], BF16, tag=f"nm{lo}")
            else:
                mid = dst[:, lo:hi, :]
            eng.tensor_tensor(
                mid[:], src_tile[:, lo:hi, :],
                rstd_b[:, None, :].to_broadcast((P, hi - lo, N)), OP.mult)
            eng.tensor_tensor(
                dst[:, lo:hi, :], mid[:],
                nmr_b[:, None, :].to_broadcast((P, hi - lo, N)), OP.add)
        return dst

    def ln1_phase(sb):
        mark(f"ln1({sb})")
        xT_s = xT_p.tile([P, NCH, N], BF16, tag="x")
        nc.sync.dma_start(xT_s[:], xT_a[:, :, sb * N:(sb + 1) * N])
        hT = layernorm(xT_s, hT_p, F8)
        return xT_s, hT

    def _qk_pair(j, hT):
        q_ps = ps_mm.tile([P, N], F32, tag="mm")
        for cp in range(NCH // 2):
            nc.tensor.matmul(q_ps[:], wq_sb[:, 2 * cp:2 * cp + 2, j * P:(j + 1) * P],
                             hT[:, 2 * cp:2 * cp + 2, :],
                             start=(cp == 0), stop=(cp == NCH // 2 - 1),
                             perf_mode=DR)
        q_sb = qk_p.tile([P, N], BF16, tag="qk")
        nc.vector.tensor_scalar(q_sb[:], q_ps[:], 1.0 / WS,
                                bq_sb[:, j:j + 1], OP.mult, OP.add)
        k_ps = ps_mm.tile([P, N], F32, tag="mm")
        for cp in range(NCH // 2):
            nc.tensor.matmul(k_ps[:], wk_sb[:, 2 * cp:2 * cp + 2, j * P:(j + 1) * P],
                             hT[:, 2 * cp:2 * cp + 2, :],
                             start=(cp == 0), stop=(cp == NCH // 2 - 1),
                             perf_mode=DR)
        k_sb = qk_p.tile([P, N], BF16, tag="qk")
        nc.vector.tensor_scalar(k_sb[:], k_ps[:], 1.0 / WS,
                                bk_sb[:, j:j + 1], OP.mult, OP.add)
        return q_sb, k_sb

    def _scores_exp(j, q_sb, k_sb):
        pt2 = pt_p.tile([P, 2, S, N], F8, tag="pt")
        for m in range(S):
            scs = [ps_sc.tile([P, N], F32, tag="sc", name=f"sc{_h}")
                   for _h in range(2)]
            # (0,0) and (64,0) row-group matmuls run concurrently
            for hh in range(2):
                h0 = hh * HD
                nc.tensor.matmul(
                    scs[hh][:],
                    k_sb[h0:h0 + HD, m * P:(m + 1) * P],
                    q_sb[h0:h0 + HD, :],
                    start=True, stop=True)
            for hh in range(2):
                nc.scalar.activation(
                    pt2[:, hh, m, :], scs[hh][:],
                    AF.Exp, bias=neg2_col[:], scale=SCALE)
        return pt2

    def _av_pair(j, pt2, v_pairs, yraw, dr1):
        for hh in range(2):
            head = 2 * j + hh
            av = ps_acc.tile([P, N], F32, tag="acc")
            for pr in range(2):
                nc.tensor.matmul(av[0:VW, :], v_pairs[pr][:, :, head, :],
                                 pt2[:, hh, 2 * pr:2 * pr + 2, :],
                                 start=(pr == 0), stop=(pr == 1),
                                 perf_mode=DR)
            # stage unnormalized out + denominator row; divide happens
            # batched once all 12 heads are in
            nc.vector.tensor_scalar_mul(yraw[hh * HD:(hh + 1) * HD, j, :],
                                        av[0:HD, :], 1.0)
            drow = rows_p.tile([1, N], F32, tag="row")
            nc.vector.tensor_scalar_mul(drow[:], av[64:65, :], 1.0)
            nc.gpsimd.dma_start(dr1[head:head + 1, :], drow[:])

    def attn_phase(sb, hT):
        mark(f"attn({sb})")
        yraw = yraw_p.tile([P, NCH, N], BF16, tag="yr")
        # engines may only address partition bases 0/32/64/96, so the 12
        # denominator rows hop through DRAM one by one and come back
        # partition-major for the batched ln->exp reciprocal
        dr1 = dram_p.tile([HEADS, N], F32, tag="drg")

        # pair 0 first so the scalar engine's exp pipeline starts while the
        # V matmuls run
        q0, k0 = _qk_pair(0, hT)
        pt_prev = _scores_exp(0, q0, k0)

        # V = h^T.T @ Wv  (token-major, fp8, + ones column per head)
        v_pairs = []
        for pr in range(2):
            v3 = v_p.tile([P, 2, HEADS, VW], F8, tag="v")
            nc.vector.memset(v3[:, :, :, 64:65], 1.0)
            for ti in range(2):
                tt = 2 * pr + ti
                pvs = [ps_mm.tile([P, N], F32, tag="mm", name=f"pv{_h}")
                       for _h in range(2)]
                for half in range(2):
                    hs = slice(half * 384, (half + 1) * 384)
                    for cp in range(NCH // 2):
                        nc.tensor.matmul(
                            pvs[half][:, 0:384],
                            hT[:, 2 * cp:2 * cp + 2, tt * P:(tt + 1) * P],
                            wv_sb[:, 2 * cp:2 * cp + 2, hs],
                            start=(cp == 0), stop=(cp == NCH // 2 - 1),
                            perf_mode=DR)
                # half 0 evacuates on scalar (idle during V), half 1 on vector
                nc.scalar.activation(
                    v3[:, ti, 0:6, 0:64],
                    pvs[0][:, 0:384].rearrange("p (h d) -> p h d", d=64),
                    AF.Copy, scale=1.0 / WS)
                nc.vector.tensor_scalar_mul(
                    v3[:, ti, 6:12, 0:64],
                    pvs[1][:, 0:384].rearrange("p (h d) -> p h d", d=64),
                    1.0 / WS)
            v_pairs.append(v3)

        # pair-pipelined: exp(j) overlaps av(j-1) and qk(j+1)
        for j in range(1, NPAIR):
            qj, kj = _qk_pair(j, hT)
            pt_j = _scores_exp(j, qj, kj)
            _av_pair(j - 1, pt_prev, v_pairs, yraw, dr1)
            pt_prev = pt_j
        _av_pair(NPAIR - 1, pt_prev, v_pairs, yraw, dr1)

        # batched softmax denominators: 1/d via scalar ln->exp chain
        dsp = ds_p.tile([HEADS, N], F32, tag="dsp")
        nc.gpsimd.dma_start(dsp[:], dr1[:])
        lnd = ds_p.tile([HEADS, N], F32, tag="ds2")
        nc.scalar.activation(lnd[:], dsp[:], AF.Ln, bias=zero_col[0:HEADS, :])
        rcp = ds_p.tile([HEADS, N], BF16, tag="dsb")
        nc.scalar.activation(rcp[:], lnd[:], AF.Exp, scale=-1.0,
                             bias=zero_col[0:HEADS, :])
        drr = dram_p.tile([HEADS, N], BF16, tag="drh")
        nc.gpsimd.dma_start(drr[:], rcp[:])
        rb = rb_p.tile([P, NCH, N], BF16, tag="rb")
        src = drr[:].rearrange("(j hh) t -> hh j t", hh=2)
        for hh in range(2):
            nc.gpsimd.dma_start(
                rb[hh * HD:(hh + 1) * HD, :, :],
                src[hh:hh + 1].to_broadcast((HD, NCH, N)))
        yT = yT_p.tile([P, NCH, N], F8, tag="y")
        for eng, lo, hi in ((nc.vector, 0, 4), (nc.gpsimd, 4, NCH)):
            eng.tensor_tensor(yT[:, lo:hi, :], yraw[:, lo:hi, :],
                              rb[:, lo:hi, :], OP.mult)
        return yT

    def proj_phase(sb, yT, xT_s):
        mark(f"proj({sb})")
        x2T = x2T_p.tile([P, NCH, N], BF16, tag="x2")
        for o in range(NCH):
            pp = ps_mm.tile([P, N], F32, tag="mm")
            for cp in range(NCH // 2):
                nc.tensor.matmul(pp[:], wp_sb[:, 2 * cp:2 * cp + 2, o * P:(o + 1) * P],
                                 yT[:, 2 * cp:2 * cp + 2, :],
                                 start=(cp == 0), stop=(cp == NCH // 2 - 1),
                                 perf_mode=DR)
            # x2 = pp/WS + x   (bp folded: asserted zero on host)
            nc.vector.scalar_tensor_tensor(
                x2T[:, o, :], pp[:], 1.0 / WS, xT_s[:, o, :], OP.mult, OP.add)
        return x2T

    def _mlp1_part(sb, h2T):
        mark(f"mlp1({sb})")
        mT = mT_p.tile([P, KCH, N], BF16, tag="m")
        for og in range(KCH // 3):
            w1t = w1_p.tile([P, 3, NCH, P], BF16, tag="w1")
            nc.sync.dma_start(
                w1t[:], w1_a[og * 3:(og + 1) * 3].rearrange("o p c m -> p o c m"))
            for oi in range(3):
                o = og * 3 + oi
                pm = ps_mm.tile([P, N], F32, tag="mm")
                for c in range(NCH):
                    nc.tensor.matmul(pm[:], w1t[:, oi, c, :], h2T[:, c, :],
                                     start=(c == 0), stop=(c == NCH - 1))
                nc.scalar.activation(mT[:, o, :], pm[:], AF.Gelu,
                                     bias=bm1_sb[:, o:o + 1])
        return mT

    def _mlp2_part(sb, mT, x2T):
        mark(f"mlp2({sb})")
        sl = slice(sb * N, (sb + 1) * N)
        for o in range(NCH):
            pm2 = ps_acc.tile([P, N], F32, tag="acc")
            for k in range(KCH):
                nc.tensor.matmul(pm2[:], w2_sb[:, k, o * P:(o + 1) * P],
                                 mT[:, k, :],
                                 start=(k == 0), stop=(k == KCH - 1))
            ot = outp.tile([P, N], F32, tag="o")
            nc.vector.scalar_tensor_tensor(
                ot[:], pm2[:], bm2_sb[:, o:o + 1], x2T[:, o, :], OP.add, OP.add)
            nc.sync.dma_start(out_a[:, o, sl], ot[:])

    # ---- software-pipelined schedule over the S blocks ----
    state = {}
    deferred = {}
    state[0] = ln1_phase(0)
    # weight loads ordered so x(1) isn't stuck behind the big W1 stream
    nc.sync.dma_start(wv_sb[:], wv_a)
    nc.sync.dma_start(wq_sb[:], wq_a)
    nc.sync.dma_start(wk_sb[:], wk_a)
    state[1] = ln1_phase(1)
    nc.sync.dma_start(wp_sb[:], wp_a)
    nc.sync.dma_start(w2_sb[:], w2_a)
    yT_cur = attn_phase(0, state[0][1])
    for sb in range(S):
        xT_s, _ = state[sb]
        x2T = proj_phase(sb, yT_cur, xT_s)
        mark(f"ln2({sb})")
        h2T = layernorm(x2T, h2T_p, BF16)
        if sb + 2 < S:
            state[sb + 2] = ln1_phase(sb + 2)
        if sb + 1 < S:
            yT_cur = attn_phase(sb + 1, state[sb + 1][1])
        if sb == S - 2:
            deferred[sb] = (_mlp1_part(sb, h2T), x2T)
        else:
            if sb == S - 1 and (S - 2) in deferred:
                mTp, x2Tp = deferred.pop(S - 2)
                _mlp2_part(S - 2, mTp, x2Tp)
            mT = _mlp1_part(sb, h2T)
            _mlp2_part(sb, mT, x2T)

    for p in reversed(list(ctx_pools.values())):
        p.release()


_CACHE = {}


def _get_bass(debug=False, reps=1):
    key = (bool(debug), reps)
    if key not in _CACHE:
        _CACHE[key] = build_bass(debug, reps)
    return _CACHE[key]


def _prep_host(x, g1, b1, Wqkv, Wp, bp, g2, b2, W1, bm1, W2, bm2):
    f32 = np.float32
    bf16 = ml_dtypes.bfloat16
    f8 = ml_dtypes.float8_e4m3fn
    g1 = np.asarray(g1, f32); b1 = np.asarray(b1, f32)
    Wqkv = np.asarray(Wqkv, f32)
    Wg = Wqkv * g1[:, None]
    bias1 = b1 @ Wqkv
    q, k, v = Wg[:, :DIM], Wg[:, DIM:2 * DIM], Wg[:, 2 * DIM:]
    bias_q, bias_k, bias_v = bias1[:DIM], bias1[DIM:2 * DIM], bias1[2 * DIM:]

    def w8(w):  # [DIM, M] -> [P, NCH, M] fp8, pre-scaled x WS
        w = np.clip(np.asarray(w, f32) * WS, -240.0, 240.0)
        return np.ascontiguousarray(
            w.reshape(NCH, P, -1).transpose(1, 0, 2).astype(f8))

    def col(b, nch=NCH):  # [nch*P] -> [P, nch]
        return np.ascontiguousarray(b.reshape(nch, P).T.astype(f32))

    g2 = np.asarray(g2, f32); b2 = np.asarray(b2, f32)
    W1 = np.asarray(W1, f32)
    W1g = W1 * g2[:, None]
    bm1_eff = np.asarray(bm1, f32) + b2 @ W1

    # W1 pre-tiled for streaming (baseline layout); W2 resident [P, KCH, DIM]
    w1h = np.ascontiguousarray(
        W1g.reshape(NCH, P, KCH, P).transpose(2, 1, 0, 3).astype(bf16))
    w2t = np.ascontiguousarray(
        np.asarray(W2, f32).reshape(KCH, P, DIM).transpose(1, 0, 2).astype(bf16))

    weights = {
        "wq": w8(q), "wk": w8(k), "wv": w8(v), "wp": w8(np.asarray(Wp, f32)),
        "w1": w1h, "w2": w2t,
        "bq": col(bias_q), "bk": col(bias_k),
        "bm1": col(bm1_eff, KCH), "bm2": col(np.asarray(bm2, f32)),
    }
    assert not np.any(bias_v), (
        "nonzero V bias not supported by this kernel build")
    assert not np.any(np.asarray(bp, f32)), (
        "nonzero proj bias not supported by this kernel build")

    x = np.asarray(x, f32)
    B = x.shape[0]
    xTs = [np.ascontiguousarray(x[c].reshape(TOK, DIM).T).astype(bf16)
           for c in range(B)]
    return weights, xTs


def kernel(x, g1, b1, Wqkv, Wp, bp, g2, b2, W1, bm1, W2, bm2, _debug=False):
    weights, xTs = _prep_host(x, g1, b1, Wqkv, Wp, bp, g2, b2, W1, bm1, W2, bm2)
    nc = _get_bass(False)
    in_maps = [dict(weights, xT=xTs[c]) for c in range(8)]
    res = run_bass_kernel_spmd(nc, in_maps, core_ids=list(range(8)))
    outs = []
    for c in range(8):
        o = res.results[c]["out"]          # [DIM, TOK]
        outs.append(np.ascontiguousarray(o.T).reshape(S, N, DIM))
    full = np.stack(outs).astype(np.float32)
    if _debug:
        return full, res
    return full
